# revision 11
# baseline (speedup 1.0000x reference)
"""ConvLSTM (peephole) Trainium2 Bass kernel — v5.

Data-parallel over batch: 8 cores, one batch element each. Per timestep the
64x64 image is processed in 8 chunks of 8 rows (512 positions) so each PSUM
tile is exactly one bank; p0 is double-buffered and p1/p2 triple-buffered
(8 banks total) so matmuls never wait on the late pointwise PSUM drains
(to_ reads p1[64:128]/p2[0:32] ~3us after each chunk's matmuls end).

Conv3x3(SAME) over concat(x, h): contraction rows are packed per kx-tap (dx)
as three weight tiles sharing one activation layout:
  XK0 bf16 [128]: x@ky0 (64) + x@ky1 (64)
  MIX bf16 [96]:  x@ky2 (64) + h-ch64:96@ky2 (32)
  HF  fp8  [128,2] DoubleRow pair: (h@ky0 96 + h-ch0:32@ky1 32,
                                    h-ch32:96@ky1 64 + h-ch0:64@ky2 64)
The i/f peephole terms (w_ci*c, w_cf*c) are folded into the PE as two extra
K=96 bf16 matmuls per chunk against a bf16 shadow of c with diagonal-ish
weights, so sigmoid(i)/sigmoid(f) read PSUM directly and the DVE loses the
4 scalar_tensor_tensor ops per group that sat on the critical path.

Gate permutation [i, f, o, g] puts gates into PSUM tiles
  p0 = i(0:96) + f(0:32), p1 = f(32:96) + o(0:64), p2 = o(64:96) + g(0:96).
All pointwise SBUF tiles are channel-aligned (partition = hidden channel);
gate-to-psum offsets are absorbed on the PSUM operand. Engine APs must stay
in aligned partition quadrants: base 0 any count, base 32 max 32, base 64
max 64, base 96 max 32.
"""

import numpy as np
import ml_dtypes

IN_CH, HID = 64, 96
B, T, H, W_SP = 8, 16, 64, 64
HP = WP = 66
PADN = HP * WP           # 4356
INT_N = H * W_SP         # 4096
NCH = 8                  # chunks per timestep
CROWS = H // NCH         # 8 rows per chunk
CN = CROWS * W_SP        # 512 positions per chunk
PITCH8 = 72 * 66         # 4752, fp8 plane pitch (16-aligned)

_CACHE = {}


def _build_nc():
    import concourse.bass as bass
    import concourse.tile as tile
    from concourse import mybir, bacc
    from contextlib import ExitStack

    f32 = mybir.dt.float32
    bf16 = mybir.dt.bfloat16
    fp8 = mybir.dt.float8e4
    AF = mybir.ActivationFunctionType
    ALU = mybir.AluOpType
    PM = mybir.MatmulPerfMode

    nc = bacc.Bacc("TRN2", target_bir_lowering=False, debug=False)

    xp = nc.dram_tensor("xp", [T, IN_CH, PADN], bf16, kind="ExternalInput").ap()
    wb0 = nc.dram_tensor("wb0", [128, 9 * 128], bf16, kind="ExternalInput").ap()
    wbm = nc.dram_tensor("wbm", [96, 9 * 128], bf16, kind="ExternalInput").ap()
    wf = nc.dram_tensor("wf", [128, 9 * 256], fp8, kind="ExternalInput").ap()
    wd = nc.dram_tensor("wd", [96, 192], bf16, kind="ExternalInput").ap()
    pp = nc.dram_tensor("pp", [128, 8], f32, kind="ExternalInput").ap()
    y = nc.dram_tensor("y", [T, HID, INT_N], bf16, kind="ExternalOutput").ap()

    with tile.TileContext(nc) as tc, ExitStack() as ctx:
        const_pool = ctx.enter_context(tc.tile_pool(name="const", bufs=1))
        tset_pool = ctx.enter_context(tc.tile_pool(name="tset", bufs=1))
        tmp_pool = ctx.enter_context(tc.tile_pool(name="tmp", bufs=3))
        hout_pool = ctx.enter_context(tc.tile_pool(name="hout", bufs=3))
        psum_pool = ctx.enter_context(tc.tile_pool(name="psum", bufs=1, space="PSUM"))

        wb0_sb = const_pool.tile([128, 9 * 128], bf16, name="wb0_sb")
        wbm_sb = const_pool.tile([96, 9 * 128], bf16, name="wbm_sb")
        wf_sb = const_pool.tile([128, 9 * 256], fp8, name="wf_sb")
        wd_sb = const_pool.tile([96, 192], bf16, name="wd_sb")
        pp_sb = const_pool.tile([128, 8], f32, name="pp_sb")
        c_sb = const_pool.tile([128, INT_N], f32, name="c_sb")
        cbf = const_pool.tile([96, INT_N], bf16, name="cbf")
        hstage = const_pool.tile([HID, PITCH8], fp8, name="hstage")

        xk0 = [tset_pool.tile([128, PADN], bf16, name=f"xk0_{p}") for p in range(2)]
        mix = [tset_pool.tile([96, PADN], bf16, name=f"mix_{p}") for p in range(2)]
        hf = [tset_pool.tile([128, 2 * PITCH8], fp8, name=f"hf_{p}") for p in range(2)]

        # wf last: it is first needed at t=1, so t=0's x loads go ahead of it
        nc.sync.dma_start(wb0_sb[:], wb0[:])
        nc.sync.dma_start(wbm_sb[:], wbm[:])
        nc.sync.dma_start(wd_sb[:], wd[:])
        nc.sync.dma_start(pp_sb[:], pp[:])

        # one-time zero init
        nc.vector.memset(c_sb[:], 0.0)
        nc.gpsimd.memset(hstage[:], 0.0)
        for p in range(2):
            nc.gpsimd.memset(hf[p][:], 0.0)
            nc.vector.memset(mix[p][64:96, :], 0.0)       # h-part plane (borders persist)
            nc.vector.memset(xk0[p][0:64, 0:66], 0.0)     # ky0 top edge
            nc.vector.memset(mix[p][0:64, PADN - 66 : PADN], 0.0)  # ky2 bottom edge

        wci_d = wd_sb[0:96, 0:128]     # c(0:96) -> p0: i(96) + f(0:32)
        wcf_d = wd_sb[0:96, 128:192]   # c(0:96) -> p1: f(32:96) at cols 0:64
        wco_a = pp_sb[0:64, 2:3]
        wco_b = pp_sb[64:96, 2:3]
        b_i = pp_sb[0:96, 3:4]
        b_f_a = pp_sb[0:32, 4:5]
        b_f_b = pp_sb[32:64, 4:5]
        b_f_c = pp_sb[64:96, 4:5]
        b_o = pp_sb[0:96, 5:6]
        b_g = pp_sb[0:96, 6:7]

        hst3 = hstage[:].rearrange("k (h w) -> k h w", w=WP)

        pending_tail = None
        for t in range(T):
            par = t % 2
            nxt = (t + 1) % 2
            xk, mx, hfp = xk0[par], mix[par], hf[par]

            # x_t loads (bf16, plane layout; ky baked as storage shift)
            nc.sync.dma_start(xk[0:64, 66:PADN], xp[t, :, 0 : PADN - 66])    # ky0
            nc.sync.dma_start(xk[64:128, 0:PADN], xp[t, :, 0:PADN])         # ky1
            nc.sync.dma_start(mx[0:64, 0 : PADN - 66], xp[t, :, 66:PADN])   # ky2
            if t == 0:
                nc.sync.dma_start(wf_sb[:], wf[:])

            xk3 = xk[:].rearrange("k (h w) -> k h w", w=WP)
            mx3 = mx[:].rearrange("k (h w) -> k h w", w=WP)
            hf4 = hfp[:].rearrange("k (m h w) -> k m h w", m=2, w=WP)

            for ch in range(NCH):
                r1 = CROWS * ch + 1
                g = ch // 2            # 16-row replication group
                half = ch % 2

                ps = [
                    psum_pool.tile([128, CN], f32, tag=f"p{co}", name=f"p{co}",
                                   bufs=(2 if co == 0 else 3))
                    for co in range(3)
                ]

                # matmuls: co order [0,1,2]; 9 conv MMs per (co, chunk) plus
                # a K=96 bf16 diag MM folding the i/f peephole into p0/p1
                for co in (0, 1, 2):
                    for dx in range(3):
                        idx = co * 3 + dx
                        first_w = 0 if t > 0 else 1
                        for w in range(first_w, 3):
                            st = (dx == 0 and w == first_w)
                            sp = (dx == 2 and w == 2)
                            if w == 0:
                                nc.tensor.matmul(
                                    ps[co][:],
                                    wf_sb[0:128, 256 * idx : 256 * (idx + 1)]
                                    .rearrange("k (m c) -> k m c", m=2),
                                    hf4[:, :, r1 : r1 + 8, dx : dx + 64],
                                    start=st, stop=sp, perf_mode=PM.DoubleRow,
                                )
                            elif w == 1:
                                nc.tensor.matmul(
                                    ps[co][:],
                                    wb0_sb[0:128, 128 * idx : 128 * (idx + 1)],
                                    xk3[:, r1 : r1 + 8, dx : dx + 64],
                                    start=st, stop=sp,
                                )
                            else:
                                nc.tensor.matmul(
                                    ps[co][:],
                                    wbm_sb[0:96, 128 * idx : 128 * (idx + 1)],
                                    mx3[0:96, r1 : r1 + 8, dx : dx + 64],
                                    start=st, stop=sp,
                                )
                        if dx == 0 and t > 0 and co < 2:
                            # peephole diag MM (reads c_{t-1} shadow); emitted
                            # after the start MM and before the stop MM
                            wdd = wci_d if co == 0 else wcf_d
                            mwid = 128 if co == 0 else 64
                            nc.tensor.matmul(
                                ps[co][0:mwid, :],
                                wdd,
                                cbf[0:96, CN * ch : CN * (ch + 1)],
                                start=False, stop=False,
                            )
                p0, p1, p2 = ps

                # ---- pointwise LSTM cell on 512 positions ----
                # i = p0[0:96], f = p0[96:128]+p1[0:32]+p1[32:64],
                # o = p1[64:128]+p2[0:32], g = p2[32:64]+[64:96]+[96:128]
                # (i/f already include the c-peephole via the diag MMs)
                si = tmp_pool.tile([HID, CN], bf16, tag="si", name="si")
                sf = tmp_pool.tile([HID, CN], bf16, tag="sf", name="sf")
                so = tmp_pool.tile([HID, CN], bf16, tag="so", name="so")
                gtm = tmp_pool.tile([HID, CN], f32, tag="gtm", name="gtm")
                gt = tmp_pool.tile([HID, CN], bf16, tag="gt", name="gt")
                to_ = tmp_pool.tile([HID, CN], f32, tag="to", name="to")
                fca = tmp_pool.tile([HID, CN], f32, tag="fca", name="fca")
                ig = tmp_pool.tile([HID, CN], f32, tag="ig", name="ig")
                th = tmp_pool.tile([HID, CN], bf16, tag="th", name="th")
                if half == 0:
                    ho2 = hout_pool.tile([HID, 2 * CN], bf16, tag="ho", name="ho")
                hov = ho2[:, CN * half : CN * (half + 1)]

                cg = c_sb[0:96, CN * ch : CN * (ch + 1)]

                # early psum drains for THIS chunk first (ScalarE reads PSUM
                # directly; DVE realigns g fragments) so the p0/p1/p2 buffers
                # release before the previous chunk's chain-tail ops, which
                # are deferred below (software-pipelined pointwise)
                nc.scalar.activation(si[:], p0[0:96, :], AF.Sigmoid, bias=b_i)
                nc.scalar.activation(sf[0:32, :], p0[96:128, :], AF.Sigmoid, bias=b_f_a)
                nc.scalar.activation(sf[32:64, :], p1[0:32, :], AF.Sigmoid, bias=b_f_b)
                nc.scalar.activation(sf[64:96, :], p1[32:64, :], AF.Sigmoid, bias=b_f_c)
                nc.vector.tensor_copy(gtm[0:32, :], p2[32:64, :])
                nc.vector.tensor_copy(gtm[32:64, :], p2[64:96, :])
                nc.vector.tensor_copy(gtm[64:96, :], p2[96:128, :])

                def tail(t=t, ch=ch, si=si, sf=sf, so=so, gtm=gtm, gt=gt,
                         to_=to_, fca=fca, ig=ig, th=th, hov=hov, cg=cg,
                         p1=p1, p2=p2):
                    nc.scalar.activation(gt[:], gtm[:], AF.Tanh, bias=b_g)
                    nc.gpsimd.tensor_mul(ig[:], si[:], gt[:])    # i * tanh(g)
                    nc.gpsimd.tensor_mul(fca[:], sf[:], cg)      # f * c_old
                    nc.gpsimd.tensor_add(cg, fca[:], ig[:])      # c_new
                    nc.vector.scalar_tensor_tensor(
                        to_[0:64, :], cg[0:64, :], wco_a, p1[64:128, :],
                        ALU.mult, ALU.add,
                    )
                    nc.vector.scalar_tensor_tensor(
                        to_[64:96, :], cg[64:96, :], wco_b, p2[0:32, :],
                        ALU.mult, ALU.add,
                    )
                    nc.scalar.activation(so[:], to_[:], AF.Sigmoid, bias=b_o)
                    nc.scalar.activation(th[:], cg, AF.Tanh)
                    nc.gpsimd.tensor_mul(hov, so[:], th[:])      # h = o*tanh(c)
                    if t + 1 < T:
                        nc.vector.tensor_copy(
                            cbf[0:96, CN * ch : CN * (ch + 1)], cg
                        )

                if pending_tail is not None:
                    pending_tail()
                pending_tail = tail

                if half == 1:
                    pending_tail()
                    pending_tail = None
                    ho = ho2
                    rr = 16 * g
                    nc.sync.dma_start(
                        y[t, :, 1024 * g : 1024 * (g + 1)], ho[:]
                    )
                    if t + 1 < T:
                        # h -> fp8 staging plane (center-aligned), then replicate
                        nc.vector.tensor_copy(
                            hst3[:, rr + 1 : rr + 17, 1:65],
                            ho[:].rearrange("k (h w) -> k h w", w=64),
                        )
                        hfn = hf[nxt]
                        a = WP * (rr + 1)
                        b_ = WP * (rr + 17)
                        # ky0 (+66), ky1 (0), ky2 (-66) storage shifts
                        nc.sync.dma_start(hfn[0:96, a + 66 : b_ + 66], hstage[:, a:b_])
                        nc.sync.dma_start(
                            hfn[96:128, a : b_], hstage[0:32, a:b_]
                        )
                        nc.sync.dma_start(
                            hfn[0:64, PITCH8 + a : PITCH8 + b_], hstage[32:96, a:b_]
                        )
                        nc.sync.dma_start(
                            hfn[64:128, PITCH8 + a - 66 : PITCH8 + b_ - 66],
                            hstage[0:64, a:b_],
                        )
                        # MIX h-part (bf16, ky2): rows rr..rr+16, interior cols
                        mxn3 = mix[nxt][:].rearrange("k (h w) -> k h w", w=WP)
                        nc.sync.dma_start(
                            mxn3[64:96, rr : rr + 16, 1:65],
                            ho[64:96, :].rearrange("k (h w) -> k h w", w=64),
                        )

    nc.compile()
    return nc


def _host_inputs(x_seq, W, b, w_ci, w_cf, w_co):
    bf16 = ml_dtypes.bfloat16
    fp8 = ml_dtypes.float8_e4m3
    # padded, pre-cast x: [B, T, IN_CH, PADN]
    xpad = np.zeros((B, T, IN_CH, HP, WP), np.float32)
    xpad[:, :, :, 1:65, 1:65] = x_seq
    xpad = xpad.reshape(B, T, IN_CH, PADN).astype(bf16)

    # gate-row permutation [i, f, o, g] (reference order: i, f, g, o)
    order = np.concatenate([
        np.arange(0, 96),      # i
        np.arange(96, 192),    # f
        np.arange(288, 384),   # o
        np.arange(192, 288),   # g
    ])

    wb0 = np.zeros((9, 128, 128), np.float32)
    wbm = np.zeros((9, 96, 128), np.float32)
    wfp = np.zeros((9, 128, 2, 128), np.float32)
    for co in range(3):
        q = order[128 * co : 128 * (co + 1)]
        for dx in range(3):
            i = co * 3 + dx
            # XK0: x@ky0 (p0:64), x@ky1 (p64:128)
            wb0[i, 0:64] = W[q][:, 0:64, 0, dx].T
            wb0[i, 64:128] = W[q][:, 0:64, 1, dx].T
            # MIX: x@ky2 (p0:64), h-ch64:96@ky2 (p64:96)
            wbm[i, 0:64] = W[q][:, 0:64, 2, dx].T
            wbm[i, 64:96] = W[q][:, 128:160, 2, dx].T
            # HF pair member0: h@ky0 (96) + h-ch0:32@ky1 (32)
            wfp[i, 0:96, 0] = W[q][:, 64:160, 0, dx].T
            wfp[i, 96:128, 0] = W[q][:, 64:96, 1, dx].T
            # member1: h-ch32:96@ky1 (64) + h-ch0:64@ky2 (64)
            wfp[i, 0:64, 1] = W[q][:, 96:160, 1, dx].T
            wfp[i, 64:128, 1] = W[q][:, 64:128, 2, dx].T

    # lhsT tiles laid out [partition, free]: free = (co*3+dx)*width + cols
    wb0 = wb0.transpose(1, 0, 2).reshape(128, 9 * 128).astype(bf16)
    wbm = wbm.transpose(1, 0, 2).reshape(96, 9 * 128).astype(bf16)
    wfp = wfp.transpose(1, 0, 2, 3).reshape(128, 9 * 256).astype(fp8)

    # peephole diag weights: wd[:, 0:128] maps c(0:96) -> i(96)+f(0:32);
    # wd[:, 128:192] maps c(0:96) -> f(32:96)
    wdd = np.zeros((96, 192), np.float32)
    wci_f = w_ci[:, 0, 0]
    wcf_f = w_cf[:, 0, 0]
    for k in range(96):
        wdd[k, k] = wci_f[k]
    for k in range(32):
        wdd[k, 96 + k] = wcf_f[k]
    for k in range(64):
        wdd[32 + k, 128 + k] = wcf_f[32 + k]
    wdd = wdd.astype(bf16)

    pp = np.zeros((128, 8), np.float32)
    pp[0:96, 0] = w_ci[:, 0, 0]
    pp[0:96, 1] = w_cf[:, 0, 0]
    pp[0:96, 2] = w_co[:, 0, 0]
    pp[0:96, 3] = b[0:96]        # b_i
    pp[0:96, 4] = b[96:192]      # b_f
    pp[0:96, 5] = b[288:384]     # b_o
    pp[0:96, 6] = b[192:288]     # b_g
    return xpad, wb0, wbm, wfp, wdd, pp


def _in_maps(inputs):
    xpad, wb0, wbm, wfp, wdd, pp = _host_inputs(
        np.asarray(inputs["x_seq"], np.float32), np.asarray(inputs["W"], np.float32),
        np.asarray(inputs["b"], np.float32), np.asarray(inputs["w_ci"], np.float32),
        np.asarray(inputs["w_cf"], np.float32), np.asarray(inputs["w_co"], np.float32),
    )
    return [
        {"xp": xpad[i], "wb0": wb0, "wbm": wbm, "wf": wfp, "wd": wdd, "pp": pp}
        for i in range(B)
    ]


def kernel(x_seq, W, b, w_ci, w_cf, w_co):
    from concourse import bass_utils

    if "nc" not in _CACHE:
        _CACHE["nc"] = _build_nc()
    nc = _CACHE["nc"]

    in_maps = _in_maps({"x_seq": x_seq, "W": W, "b": b, "w_ci": w_ci,
                        "w_cf": w_cf, "w_co": w_co})

    last = None
    for _ in range(3):  # retry: first exec after a wedged device can flake
        try:
            res = bass_utils.run_bass_kernel_spmd(nc, in_maps, list(range(B)))
            break
        except Exception as e:  # noqa: BLE001
            last = e
    else:
        raise last

    out = np.stack(
        [res.results[i]["y"].reshape(T, HID, H, W_SP) for i in range(B)], axis=0
    )
    return out.astype(np.float32)
